# revision 1
# baseline (speedup 1.0000x reference)
"""Trainium2 Bass kernel for nn_CrossAttention (B=2, N=2048, D=1024, H=16).

Sharding (8 cores): core c -> (batch b = c//4, head-group hg = c%4).
Each head-group is 4 heads = 256 of the 1024 projection dims.

Per core:
  QT = (Wq_hg @ X_b^T)      [256, 2048]   (transposed projections)
  KT = (Wk_hg @ X_b^T)      [256, 2048]
  V  = (X_b @ Wv_hg^T)      [2048, 256]   (natural orientation, + ones col)
  per head h, q-tile: ST = KT_h_tile^T-style scores [tok_k, tok_q] on PE,
  exp on ScalarE (scale folded, no max subtraction: logits ~ N(0,1)),
  PV matmul with ones-augmented V gives x^T and softmax denominators,
  out_partial = x_hg @ Wo_hg^T + bo/4   [2048, 1024]
  ReduceScatter(add) over the 4 cores of the batch -> each core owns a
  disjoint 512-token slice of the final output; host concatenates.

All matmuls run as float32r (FP22 reduced precision, 1 cycle/row on PE).
"""

import numpy as np

B = 2
NT = 2048
D = 1024
HEADS = 16
DH = 64
NCORES = 8
CPB = 4  # cores per batch
HG = HEADS // NCORES * 2  # 4 heads per core
HGD = HG * DH  # 256 cols per core
GROUPS = [[0, 1, 2, 3], [4, 5, 6, 7]]
SCALE = DH ** -0.5

_patched = False


def _patch_tile_drain():
    """This container's walrus rejects >1 sync-wait on a Drain
    (CoreV3GenImpl setupSyncWait<CTRL_NO_STRUCT>: "Too many sync wait
    commands").  Split the final TileContext drain's waits across a chain
    of single-wait drains; semaphores are monotonic so sequential waits
    are equivalent to one multi-wait."""
    global _patched
    if _patched:
        return
    import concourse.tile as tile
    import concourse.mybir as mybir
    from concourse.vector_clock import ScopedClock

    _uid = [0]

    def _split_multiwaits(nc):
        # Walrus here allows only ONE sync-wait per instruction; hoist
        # extra waits onto single-wait NoOps inserted just before, on the
        # same engine (engine execution is serial, sems are monotonic).
        for f in nc.m.functions:
            for bb in f.blocks:
                il = bb.instructions
                i = 0
                while i < len(il):
                    inst = il[i]
                    si = inst.sync_info
                    if si is not None and len(si.on_wait) > 1:
                        waits = list(si.on_wait)
                        inst.sync_info = mybir.SyncInfo(
                            on_wait=[waits[-1]], on_update=list(si.on_update)
                        )
                        for w in waits[:-1]:
                            _uid[0] += 1
                            nop = mybir.InstEventSemaphore(
                                name=f"WSPLIT-{_uid[0]}",
                                engine=inst.engine,
                                ins=[],
                                outs=[],
                                sync_info=mybir.SyncInfo(
                                    on_wait=[w], on_update=[]),
                            )
                            il.insert(i, nop)
                            i += 1
                    i += 1

    def _drain_and_barrier(self, tick_clock, wait_clock):
        nc = self.nc
        drain_inst = nc.sync.drain()
        wait_clock.add_sem_waits(
            drain_inst.ins, ScopedClock({None: tick_clock.global_clock})
        )
        si = drain_inst.ins.sync_info
        if si is not None and len(si.on_wait) > 1:
            waits = list(si.on_wait)
            drain_inst.ins.sync_info = mybir.SyncInfo(
                on_wait=[waits[0]], on_update=list(si.on_update)
            )
            for w in waits[1:]:
                extra = nc.sync.drain()
                extra.ins.sync_info = mybir.SyncInfo(on_wait=[w], on_update=[])

        _split_multiwaits(nc)
        nc.all_engine_barrier()
        assert self.sems is not None
        popped = nc._tile_sem_poison_stack.pop()
        assert popped is self._sem_poison
        nc.clear_and_free_semaphores(list(self.sems.allocated().values()))
        nc.all_engine_barrier()

    tile.TileContext._drain_and_barrier = _drain_and_barrier
    _patched = True


def build_program(nt=NT):
    """Build the SPMD Bass program (one NeuronCore's view)."""
    _patch_tile_drain()
    import concourse.bass as bass
    import concourse.tile as tile
    import concourse.mybir as mybir

    f32 = mybir.dt.float32
    f32r = mybir.dt.float32r
    EXP = mybir.ActivationFunctionType.Exp

    NQT = nt // 512   # q tiles (rhs free dim 512)
    NKT = nt // 128   # k tiles (PE contraction dim 128)
    NMT = nt // 128   # token m-tiles
    QSL = nt // CPB   # output token slice per core

    nc = bass.Bass("TRN2", target_bir_lowering=False, debug=False,
                   num_devices=NCORES)

    xqT = nc.dram_tensor("xqT", [D, nt], f32r, kind="ExternalInput")
    xkT = nc.dram_tensor("xkT", [D, nt], f32r, kind="ExternalInput")
    xvT = nc.dram_tensor("xvT", [D, nt], f32r, kind="ExternalInput")
    wqT = nc.dram_tensor("wqT", [D, HGD], f32r, kind="ExternalInput")
    wkT = nc.dram_tensor("wkT", [D, HGD], f32r, kind="ExternalInput")
    wvT = nc.dram_tensor("wvT", [D, HGD], f32r, kind="ExternalInput")
    woT = nc.dram_tensor("woT", [HGD, D], f32r, kind="ExternalInput")
    bo4 = nc.dram_tensor("bo4", [D], f32, kind="ExternalInput")
    out = nc.dram_tensor("out", [QSL, D], f32, kind="ExternalOutput")

    partial = nc.dram_tensor("partial", [nt, D], f32)
    rsout = nc.dram_tensor("rsout", [QSL, D], f32)
    rbounce = nc.dram_tensor("rbounce", [16, 512], f32)

    with tile.TileContext(nc) as tc:
        from contextlib import ExitStack
        with ExitStack() as ctx:
            const = ctx.enter_context(tc.tile_pool(name="const", bufs=1))
            persist = ctx.enter_context(tc.tile_pool(name="persist", bufs=1))
            rhs_pool = ctx.enter_context(tc.tile_pool(name="rhs", bufs=4))
            pt_pool = ctx.enter_context(tc.tile_pool(name="pt", bufs=4))
            misc = ctx.enter_context(tc.tile_pool(name="misc", bufs=4))
            outsb = ctx.enter_context(tc.tile_pool(name="outsb", bufs=3))
            # PSUM: 8 banks of [128, 512]f32 total.  One shared 4-slot pool
            # for all plain matmul accumulators ("mm" tag), 2 slots for the
            # attention x^T accumulators, 2 for the output projection.
            st_ps = ctx.enter_context(
                tc.tile_pool(name="st_ps", bufs=4, space="PSUM"))
            xa_ps = ctx.enter_context(
                tc.tile_pool(name="xa_ps", bufs=2, space="PSUM"))
            op_ps = ctx.enter_context(
                tc.tile_pool(name="op_ps", bufs=2, space="PSUM"))

            # --- constants -------------------------------------------------
            wq_sb = const.tile([128, 8, HGD], f32r)   # [k-part, k-tile, col]
            wk_sb = const.tile([128, 8, HGD], f32r)
            wv_sb = const.tile([128, 8, HGD], f32r)
            wo_sb = const.tile([128, 2, D], f32r)     # [d-part, hg k-tile, odim]
            bias_sb = const.tile([128, D], f32)      # bo/4 broadcast over parts
            nc.sync.dma_start(out=wq_sb[:], in_=wqT[:].rearrange(
                "(t p) c -> p t c", p=128))
            nc.sync.dma_start(out=wk_sb[:], in_=wkT[:].rearrange(
                "(t p) c -> p t c", p=128))
            nc.sync.dma_start(out=wv_sb[:], in_=wvT[:].rearrange(
                "(t p) c -> p t c", p=128))
            nc.sync.dma_start(out=wo_sb[:], in_=woT[:].rearrange(
                "(t p) c -> p t c", p=128))
            nc.sync.dma_start(out=bias_sb[:],
                              in_=bo4[:].partition_broadcast(128))
            ones_sb = const.tile([128, 1], f32)
            nc.vector.memset(ones_sb[:], 1.0)

            # --- persistent activations -----------------------------------
            qt_sb = persist.tile([128, 2, nt], f32r)   # QT: [qcol%128, qcol//128, tok]
            kt_sb = persist.tile([128, 2, nt], f32r)
            v_sb = persist.tile([128, NMT, HG * (DH + 1)], f32r)  # + ones col
            xt_sb = persist.tile([128, 2, nt], f32r)   # x^T (normalized)

            # --- Q/K projections: out[qcol, tok] = sum_k W^T[k,qcol] X^T[k,tok]
            for (src, wsb, dst) in ((xqT, wq_sb, qt_sb), (xkT, wk_sb, kt_sb)):
                for n in range(NQT):
                    acc = [st_ps.tile([128, 512], f32, tag="mm", name="qkacc")
                           for _ in range(2)]
                    for k in range(8):
                        rhs = rhs_pool.tile([128, 512], f32r, tag="projrhs")
                        nc.sync.dma_start(
                            out=rhs[:],
                            in_=src[128 * k:128 * (k + 1),
                                    512 * n:512 * (n + 1)])
                        for m in range(2):
                            nc.tensor.matmul(
                                acc[m][:],
                                wsb[:, k, 128 * m:128 * (m + 1)],
                                rhs[:],
                                start=(k == 0), stop=(k == 7))
                    for m in range(2):
                        nc.vector.tensor_copy(
                            dst[:, m, 512 * n:512 * (n + 1)], acc[m][:])

            # --- V projection: out[tok, vcol] = sum_k X^T[k,tok] W^T[k,vcol]
            for mg in range(nt // 512):
                acc = [st_ps.tile([128, 512], f32, tag="mm", name="vacc")
                       for _ in range(4)]
                for k in range(8):
                    lhs = rhs_pool.tile([128, 512], f32r, tag="projrhs")
                    nc.sync.dma_start(
                        out=lhs[:],
                        in_=xvT[128 * k:128 * (k + 1),
                                512 * mg:512 * (mg + 1)])
                    for m in range(4):
                        nc.tensor.matmul(
                            acc[m][:, 0:HGD],
                            lhs[:, 128 * m:128 * (m + 1)],
                            wv_sb[:, k, :],
                            start=(k == 0), stop=(k == 7))
                for m in range(4):
                    mt = 4 * mg + m
                    for h in range(HG):
                        nc.vector.tensor_copy(
                            v_sb[:, mt, (DH + 1) * h:(DH + 1) * h + DH],
                            acc[m][:, DH * h:DH * (h + 1)])
                        nc.vector.tensor_copy(
                            v_sb[:, mt, (DH + 1) * h + DH:(DH + 1) * (h + 1)],
                            ones_sb[:])

            # --- attention + output projection, per q-tile ----------------
            for qt in range(NQT):
                qsl = slice(512 * qt, 512 * (qt + 1))
                for hp in range(2):
                    xa = [xa_ps.tile([DH + 1, 512], f32, tag="xa", name="xa")
                          for _ in range(2)]
                    for kt in range(NKT):
                        st = [st_ps.tile([128, 512], f32, tag="mm", name="st")
                              for _ in range(2)]
                        for j in range(2):
                            p0 = 64 * j
                            nc.tensor.matmul(
                                st[j][:],
                                kt_sb[p0:p0 + 64, hp,
                                      128 * kt:128 * (kt + 1)],
                                qt_sb[p0:p0 + 64, hp, qsl],
                                tile_position=(p0, 0))
                        for j in range(2):
                            h = 2 * hp + j
                            pt = pt_pool.tile([128, 512], f32r)
                            nc.scalar.activation(pt[:], st[j][:], EXP,
                                                 scale=SCALE)
                            nc.tensor.matmul(
                                xa[j][:],
                                v_sb[:, kt,
                                     (DH + 1) * h:(DH + 1) * (h + 1)
                                     ],
                                pt[:],
                                start=(kt == 0), stop=(kt == NKT - 1))
                    # normalize: x[d, tok] /= sums[tok] (sums in row DH)
                    for j in range(2):
                        rr = misc.tile([DH + 1, 512], f32, tag="rr")
                        nc.vector.reciprocal(rr[DH:DH + 1, :],
                                             xa[j][DH:DH + 1, :])
                        bc = misc.tile([DH, 512], f32, tag="bc")
                        rb = rbounce[(4 * (qt % 2) + 2 * hp + j)
                                     % 16:(4 * (qt % 2) + 2 * hp + j) % 16 + 1,
                                     :]
                        nc.sync.dma_start(out=rb, in_=rr[DH:DH + 1, :])
                        nc.sync.dma_start(out=bc[:],
                                          in_=rb.partition_broadcast(DH))
                        if j == 0:
                            nc.vector.tensor_mul(
                                xt_sb[0:DH, hp, qsl], xa[j][0:DH, :], bc[:])
                        else:
                            tm = misc.tile([DH, 512], f32r, tag="tm")
                            nc.vector.tensor_mul(tm[:], xa[j][0:DH, :], bc[:])
                            nc.sync.dma_start(out=xt_sb[DH:128, hp, qsl],
                                              in_=tm[:])

                # out-proj for this q-tile:
                # partial[t, o] = sum_d x^T[d, t] Wo^T[d, o]  (+ bo/4)
                for n in range(2):
                    osl = slice(512 * n, 512 * (n + 1))
                    for m in range(4):
                        tsl = slice(512 * qt + 128 * m,
                                    512 * qt + 128 * (m + 1))
                        acc = op_ps.tile([128, 512], f32, tag="op")
                        for k in range(2):
                            nc.tensor.matmul(
                                acc[:],
                                xt_sb[:, k, tsl],
                                wo_sb[:, k, osl],
                                start=(k == 0), stop=(k == 1))
                        ob = outsb.tile([128, 512], f32)
                        nc.vector.tensor_add(ob[:], acc[:], bias_sb[:, osl])
                        nc.sync.dma_start(out=partial[tsl, osl], in_=ob[:])

            # --- reduce-scatter over the 4 cores of this batch ------------
            cc = nc.gpsimd.collective_compute(
                "ReduceScatter",
                mybir.AluOpType.add,
                replica_groups=GROUPS,
                ins=[partial[:]],
                outs=[rsout[:]],
            )
            fin = nc.sync.dma_start(out=out[:], in_=rsout[:])
            tile.add_dep_helper(fin.ins, cc.ins, reason="out after RS")

    return nc


_CACHE = {}


def _get_program(nt=NT):
    if nt not in _CACHE:
        _CACHE[nt] = build_program(nt)
    return _CACHE[nt]


def make_in_maps(query, key, value, Wq, Wk, Wv, Wo, bo, nt=NT):
    """Host-side sharding: per-core input dicts."""
    query = np.asarray(query, dtype=np.float32)
    key = np.asarray(key, dtype=np.float32)
    value = np.asarray(value, dtype=np.float32)
    Wq = np.asarray(Wq, dtype=np.float32)
    Wk = np.asarray(Wk, dtype=np.float32)
    Wv = np.asarray(Wv, dtype=np.float32)
    Wo = np.asarray(Wo, dtype=np.float32)
    bo = np.asarray(bo, dtype=np.float32)

    xT = [np.ascontiguousarray(x.T) for x in
          (query[0], key[0], value[0], query[1], key[1], value[1])]
    bo4 = (bo * 0.25).astype(np.float32)
    in_maps = []
    for c in range(NCORES):
        b, hg = divmod(c, CPB)
        hsl = slice(HGD * hg, HGD * (hg + 1))
        in_maps.append({
            "xqT": xT[3 * b + 0],
            "xkT": xT[3 * b + 1],
            "xvT": xT[3 * b + 2],
            "wqT": np.ascontiguousarray(Wq[hsl, :].T),
            "wkT": np.ascontiguousarray(Wk[hsl, :].T),
            "wvT": np.ascontiguousarray(Wv[hsl, :].T),
            "woT": np.ascontiguousarray(Wo[:, hsl].T),
            "bo4": bo4,
        })
    return in_maps


def assemble(results, nt=NT):
    """Concatenate per-core disjoint token slices into [B, NT, D]."""
    out = np.empty((B, nt, D), dtype=np.float32)
    qsl = nt // CPB
    for c in range(NCORES):
        b, p = divmod(c, CPB)
        out[b, qsl * p:qsl * (p + 1), :] = results[c]["out"]
    return out


def run(query, key, value, Wq, Wk, Wv, Wo, bo, nt=NT, trace=False):
    from concourse.bass_utils import run_bass_kernel_spmd
    nc = _get_program(nt)
    in_maps = make_in_maps(query, key, value, Wq, Wk, Wv, Wo, bo, nt=nt)
    res = run_bass_kernel_spmd(nc, in_maps, core_ids=list(range(NCORES)),
                               trace=trace)
    return assemble(res.results, nt=nt), res


def kernel(query, key, value, qpos=None, kpos=None, Wq=None, Wk=None,
           Wv=None, Wo=None, bo=None):
    out, _ = run(query, key, value, Wq, Wk, Wv, Wo, bo)
    return out



# revision 7
# speedup vs baseline: 1.8261x; 1.8261x over previous
"""Trainium2 Bass kernel for nn_CrossAttention (B=2, N=2048, D=1024, H=16).

Token-sharded design (8 cores): core c -> (batch b = c//4, q-slice g = c%4).
Each core owns 512 query tokens of one batch and computes ALL 16 heads for
them, so its output [512, 1024] is complete locally - no output collective
(the baseline's 8MB fp32 ReduceScatter cost ~115us of exposed tail).

K/V projections are sharded by head-group (core with g computes heads
4g..4g+3 over all 2048 tokens) and AllGathered (bf16, 1MB per rank each)
within the 4-core batch group, overlapped with the Q projection.

All matmuls run in bf16 with fp32 PSUM accumulation (rel err ~6e-3 vs the
fp32 reference; tolerance 2e-2).  Attention per head pair (even head on PE
rows 0-63, odd on 64-127 via tile_position): scores^T for both heads land
in one 2-bank [128, 1024] PSUM tile, one ACT Exp instruction covers the
pair, then two PV matmuls with ones-augmented V accumulate x^T plus the
softmax denominators.  Normalization: reciprocal_approx_fast on DVE
straight from PSUM, DRAM bounce for the partition broadcast, DVE multiply.
"""

import numpy as np

B = 2
NT = 2048
D = 1024
HEADS = 16
DH = 64
NCORES = 8
CPB = 4            # cores per batch
QSL = NT // CPB    # 512 own query tokens per core
HG = HEADS // CPB  # 4 heads per k/v shard group
HGD = HG * DH      # 256 projection dims per group
VW = DH + 1        # V columns per head incl. ones column
GROUPS = [[0, 1, 2, 3], [4, 5, 6, 7]]
SCALE = DH ** -0.5
NKT = NT // 128    # 16 key-token tiles

_patched = False


def _patch_tile_drain():
    """This container's walrus rejects >1 sync-wait on a Drain
    (CoreV3GenImpl setupSyncWait<CTRL_NO_STRUCT>: "Too many sync wait
    commands").  Split the final TileContext drain's waits across a chain
    of single-wait drains; semaphores are monotonic so sequential waits
    are equivalent to one multi-wait."""
    global _patched
    if _patched:
        return
    import concourse.tile as tile
    import concourse.mybir as mybir
    from concourse.vector_clock import ScopedClock

    _uid = [0]

    def _split_multiwaits(nc):
        for f in nc.m.functions:
            for bb in f.blocks:
                il = bb.instructions
                i = 0
                while i < len(il):
                    inst = il[i]
                    si = inst.sync_info
                    if si is not None and len(si.on_wait) > 1:
                        waits = list(si.on_wait)
                        inst.sync_info = mybir.SyncInfo(
                            on_wait=[waits[-1]], on_update=list(si.on_update)
                        )
                        for w in waits[:-1]:
                            _uid[0] += 1
                            nop = mybir.InstEventSemaphore(
                                name=f"WSPLIT-{_uid[0]}",
                                engine=inst.engine,
                                ins=[],
                                outs=[],
                                sync_info=mybir.SyncInfo(
                                    on_wait=[w], on_update=[]),
                            )
                            il.insert(i, nop)
                            i += 1
                    i += 1

    def _drain_and_barrier(self, tick_clock, wait_clock):
        nc = self.nc
        drain_inst = nc.sync.drain()
        wait_clock.add_sem_waits(
            drain_inst.ins, ScopedClock({None: tick_clock.global_clock})
        )
        si = drain_inst.ins.sync_info
        if si is not None and len(si.on_wait) > 1:
            waits = list(si.on_wait)
            drain_inst.ins.sync_info = mybir.SyncInfo(
                on_wait=[waits[0]], on_update=list(si.on_update)
            )
            for w in waits[1:]:
                extra = nc.sync.drain()
                extra.ins.sync_info = mybir.SyncInfo(on_wait=[w], on_update=[])

        _split_multiwaits(nc)
        nc.all_engine_barrier()
        assert self.sems is not None
        popped = nc._tile_sem_poison_stack.pop()
        assert popped is self._sem_poison
        nc.clear_and_free_semaphores(list(self.sems.allocated().values()))
        nc.all_engine_barrier()

    tile.TileContext._drain_and_barrier = _drain_and_barrier
    _patched = True


def build_program():
    """Build the SPMD Bass program (one NeuronCore's view)."""
    _patch_tile_drain()
    import concourse.bass as bass
    import concourse.tile as tile
    import concourse.mybir as mybir

    f32 = mybir.dt.float32
    bf16 = mybir.dt.bfloat16
    EXP = mybir.ActivationFunctionType.Exp

    nc = bass.Bass("TRN2", target_bir_lowering=False, debug=False,
                   num_devices=NCORES)

    xqT = nc.dram_tensor("xqT", [D, QSL], bf16, kind="ExternalInput")
    xkT = nc.dram_tensor("xkT", [D, NT], bf16, kind="ExternalInput")
    xvT = nc.dram_tensor("xvT", [D, NT], bf16, kind="ExternalInput")
    wqT = nc.dram_tensor("wqT", [D, D], bf16, kind="ExternalInput")
    wkT = nc.dram_tensor("wkT", [D, HGD], bf16, kind="ExternalInput")
    wvT = nc.dram_tensor("wvT", [D, HGD], bf16, kind="ExternalInput")
    woT = nc.dram_tensor("woT", [D, D], bf16, kind="ExternalInput")
    bo = nc.dram_tensor("bo", [D], f32, kind="ExternalInput")
    out = nc.dram_tensor("out", [QSL, D], f32, kind="ExternalOutput")

    k_src = nc.dram_tensor("k_src", [HGD, NT], bf16)
    v_src = nc.dram_tensor("v_src", [NT, HGD], bf16)
    k_all = nc.dram_tensor("k_all", [CPB, HGD, NT], bf16)
    v_all = nc.dram_tensor("v_all", [CPB, NT, HGD], bf16)
    rbounce = nc.dram_tensor("rbounce", [4, 2 * QSL], f32)

    with tile.TileContext(nc) as tc:
        from contextlib import ExitStack
        with ExitStack() as ctx:
            const = ctx.enter_context(tc.tile_pool(name="const", bufs=1))
            persist = ctx.enter_context(tc.tile_pool(name="persist", bufs=1))
            rhs_pool = ctx.enter_context(tc.tile_pool(name="rhs", bufs=6))
            pt_pool = ctx.enter_context(tc.tile_pool(name="pt", bufs=4))
            misc = ctx.enter_context(tc.tile_pool(name="misc", bufs=4))
            outsb = ctx.enter_context(tc.tile_pool(name="outsb", bufs=3))
            # PSUM: 8 banks of [128, 512]f32.  big_ps holds 2-bank
            # [128, 1024] tiles (score pairs; also proj/out-proj accs),
            # xa_ps holds 1-bank tiles (attention x^T accs, V-proj accs).
            big_ps = ctx.enter_context(
                tc.tile_pool(name="big_ps", bufs=2, space="PSUM"))
            xa_ps = ctx.enter_context(
                tc.tile_pool(name="xa_ps", bufs=4, space="PSUM"))

            # --- warm the Exp activation table during the DMA ramp ---------
            junk = const.tile([1, 8], f32)
            nc.vector.memset(junk[:], 0.0)
            jout = const.tile([1, 8], bf16)
            nc.scalar.activation(jout[:], junk[:], EXP)

            # --- constants -------------------------------------------------
            wk_sb = const.tile([128, 8, HGD], bf16)   # [d-part, d-tile, col]
            wv_sb = const.tile([128, 8, HGD], bf16)
            wq_sb = const.tile([128, 8, D], bf16)
            wo_sb = const.tile([128, 8, D], bf16)
            bias_sb = const.tile([128, D], f32)
            xq_sb = const.tile([128, 8, QSL], bf16)
            nc.sync.dma_start(out=wk_sb[:], in_=wkT[:].rearrange(
                "(t p) c -> p t c", p=128))
            nc.sync.dma_start(out=wv_sb[:], in_=wvT[:].rearrange(
                "(t p) c -> p t c", p=128))
            nc.sync.dma_start(out=xq_sb[:], in_=xqT[:].rearrange(
                "(t p) c -> p t c", p=128))
            nc.sync.dma_start(out=wq_sb[:], in_=wqT[:].rearrange(
                "(t p) c -> p t c", p=128))
            nc.sync.dma_start(out=wo_sb[:], in_=woT[:].rearrange(
                "(t p) c -> p t c", p=128))
            nc.sync.dma_start(out=bias_sb[:],
                              in_=bo[:].partition_broadcast(128))

            # --- persistent activations -----------------------------------
            # kt_sb block t holds K^T rows [128t, 128t+128) = head pair t
            kt_sb = persist.tile([128, 8, NT], bf16)
            qt_sb = persist.tile([128, 8, QSL], bf16)
            # v_sb: [k-token part, k-tile, head * (DH cols | ones col)]
            v_sb = persist.tile([128, NKT, HEADS * VW], bf16)
            xt_sb = persist.tile([128, 8, QSL], bf16)   # x^T normalized

            # ones columns of v_sb, all heads at once
            nc.vector.memset(
                v_sb[:].rearrange("p m (h c) -> p (m h) c", c=VW)[:, :, DH:],
                1.0)

            # --- K projection (own head group, all 2048 tokens) -----------
            # KT[qdim, tok] = sum_d Wk^T[d, qdim] X_k^T[d, tok]
            ksrc_dmas = []
            for n in range(4):
                nsl = slice(512 * n, 512 * (n + 1))
                acc = [big_ps.tile([128, 1024], f32, tag="mm", name="kacc")
                       for _ in range(2)]
                for k in range(8):
                    rhs = rhs_pool.tile([128, 512], bf16, tag="projrhs")
                    nc.sync.dma_start(
                        out=rhs[:],
                        in_=xkT[128 * k:128 * (k + 1), nsl])
                    for m in range(2):
                        nc.tensor.matmul(
                            acc[m][:, 0:512],
                            wk_sb[:, k, 128 * m:128 * (m + 1)],
                            rhs[:],
                            start=(k == 0), stop=(k == 7))
                for m in range(2):
                    kslab = misc.tile([128, 512], bf16, tag="kslab")
                    nc.vector.tensor_copy(kslab[:], acc[m][:, 0:512])
                    d = nc.sync.dma_start(
                        out=k_src[128 * m:128 * (m + 1), nsl], in_=kslab[:])
                    ksrc_dmas.append(d)

            cc_k = nc.gpsimd.collective_compute(
                "AllGather", mybir.AluOpType.bypass,
                replica_groups=GROUPS,
                ins=[k_src[:]], outs=[k_all[:]])
            for d in ksrc_dmas:
                tile.add_dep_helper(cc_k.ins, d.ins, reason="agk in")

            # --- V projection (own head group) ----------------------------
            # V[tok, vcol] = sum_d X_v^T[d, tok] Wv^T[d, vcol]
            vsrc_dmas = []
            for mg in range(4):
                acc = [xa_ps.tile([128, HGD], f32, tag="xa", name="vacc")
                       for _ in range(4)]
                for k in range(8):
                    lhs = rhs_pool.tile([128, 512], bf16, tag="projrhs")
                    nc.sync.dma_start(
                        out=lhs[:],
                        in_=xvT[128 * k:128 * (k + 1),
                                512 * mg:512 * (mg + 1)])
                    for m in range(4):
                        nc.tensor.matmul(
                            acc[m][:],
                            lhs[:, 128 * m:128 * (m + 1)],
                            wv_sb[:, k, :],
                            start=(k == 0), stop=(k == 7))
                for m in range(4):
                    mt = 4 * mg + m
                    vslab = misc.tile([128, HGD], bf16, tag="vslab")
                    nc.vector.tensor_copy(vslab[:], acc[m][:])
                    d = nc.sync.dma_start(
                        out=v_src[128 * mt:128 * (mt + 1), :], in_=vslab[:])
                    vsrc_dmas.append(d)

            cc_v = nc.gpsimd.collective_compute(
                "AllGather", mybir.AluOpType.bypass,
                replica_groups=GROUPS,
                ins=[v_src[:]], outs=[v_all[:]])
            for d in vsrc_dmas:
                tile.add_dep_helper(cc_v.ins, d.ins, reason="agv in")

            # --- Q projection (own 512 tokens, all head dims) -------------
            for blk in range(8):
                acc = big_ps.tile([128, 1024], f32, tag="mm", name="qacc")
                for k in range(8):
                    nc.tensor.matmul(
                        acc[:, 0:512],
                        wq_sb[:, k, 128 * blk:128 * (blk + 1)],
                        xq_sb[:, k, :],
                        start=(k == 0), stop=(k == 7))
                nc.vector.tensor_copy(qt_sb[:, blk, :], acc[:, 0:512])

            # --- load gathered K/V (own slice round-trips the gather) -----
            for t in range(8):
                d = nc.sync.dma_start(
                    out=kt_sb[:, t, :],
                    in_=k_all[t // 2, 128 * (t % 2):128 * (t % 2 + 1), :])
                tile.add_dep_helper(d.ins, cc_k.ins, reason="k after agk")
            for r in range(CPB):
                for j in range(HG):
                    h = HG * r + j
                    d = nc.sync.dma_start(
                        out=v_sb[:].rearrange(
                            "p m (h c) -> p m h c", c=VW)[:, :, h, 0:DH],
                        in_=v_all[r, :, DH * j:DH * (j + 1)].rearrange(
                            "(m p) c -> p m c", p=128))
                    tile.add_dep_helper(d.ins, cc_v.ins, reason="v after agv")

            # --- attention, one head pair (= kt_sb block) at a time -------
            for blk in range(8):
                he, ho = 2 * blk, 2 * blk + 1
                xa_e = xa_ps.tile([VW, 512], f32, tag="xa", name="xa")
                xa_o = xa_ps.tile([VW, 512], f32, tag="xa", name="xa")
                for kt in range(NKT):
                    ksl = slice(128 * kt, 128 * (kt + 1))
                    st = big_ps.tile([128, 1024], f32, tag="mm", name="st")
                    nc.tensor.matmul(
                        st[:, 0:512],
                        kt_sb[0:64, blk, ksl], qt_sb[0:64, blk, :],
                        tile_position=(0, 0))
                    nc.tensor.matmul(
                        st[:, 512:1024],
                        kt_sb[64:128, blk, ksl], qt_sb[64:128, blk, :],
                        tile_position=(64, 0))
                    pt = pt_pool.tile([128, 1024], bf16)
                    nc.scalar.activation(pt[:], st[:], EXP, scale=SCALE)
                    nc.tensor.matmul(
                        xa_e[:], v_sb[:, kt, VW * he:VW * (he + 1)],
                        pt[:, 0:512],
                        start=(kt == 0), stop=(kt == NKT - 1))
                    nc.tensor.matmul(
                        xa_o[:], v_sb[:, kt, VW * ho:VW * (ho + 1)],
                        pt[:, 512:1024],
                        start=(kt == 0), stop=(kt == NKT - 1))
                # normalize: x[d, q] /= den[q]  (den sits in row DH)
                rcp = misc.tile([1, 2 * QSL], f32, tag="rcp")
                nc.vector.reciprocal(rcp[:, 0:512], xa_e[DH:VW, :])
                nc.vector.reciprocal(rcp[:, 512:1024], xa_o[DH:VW, :])
                rb = rbounce[blk % 4:blk % 4 + 1, :]
                nc.sync.dma_start(out=rb, in_=rcp[:])
                bc_e = misc.tile([DH, 512], f32, tag="bc")
                bc_o = misc.tile([DH, 512], f32, tag="bc")
                nc.sync.dma_start(
                    out=bc_e[:],
                    in_=rb[:, 0:512].partition_broadcast(DH))
                nc.sync.dma_start(
                    out=bc_o[:],
                    in_=rb[:, 512:1024].partition_broadcast(DH))
                nc.vector.tensor_mul(
                    xt_sb[0:DH, blk, :], xa_e[0:DH, :], bc_e[:])
                tm = misc.tile([DH, 512], bf16, tag="tm")
                nc.vector.tensor_mul(tm[:], xa_o[0:DH, :], bc_o[:])
                nc.sync.dma_start(out=xt_sb[DH:128, blk, :], in_=tm[:])

            # --- output projection + bias ---------------------------------
            # out[tok, o] = sum_x x^T[x, tok] Wo^T[x, o] + bo[o]
            for m in range(4):
                tsl = slice(128 * m, 128 * (m + 1))
                for n in range(2):
                    osl = slice(512 * n, 512 * (n + 1))
                    acc = big_ps.tile([128, 1024], f32, tag="mm", name="op")
                    for k in range(8):
                        nc.tensor.matmul(
                            acc[:, 0:512],
                            xt_sb[:, k, tsl],
                            wo_sb[:, k, osl],
                            start=(k == 0), stop=(k == 7))
                    ob = outsb.tile([128, 512], f32)
                    nc.vector.tensor_add(ob[:], acc[:, 0:512],
                                         bias_sb[:, osl])
                    nc.sync.dma_start(out=out[tsl, osl], in_=ob[:])

    return nc


_CACHE = {}


def _get_program():
    if "nc" not in _CACHE:
        _CACHE["nc"] = build_program()
    return _CACHE["nc"]


def make_in_maps(query, key, value, Wq, Wk, Wv, Wo, bo):
    """Host-side sharding: per-core input dicts (bf16)."""
    import ml_dtypes
    bf = ml_dtypes.bfloat16

    def b(x):
        return np.ascontiguousarray(np.asarray(x, dtype=np.float32)
                                    .astype(bf))

    query = np.asarray(query, dtype=np.float32)
    wqT = b(np.asarray(Wq, dtype=np.float32).T)
    woT = b(np.asarray(Wo, dtype=np.float32).T)
    bo32 = np.ascontiguousarray(np.asarray(bo, dtype=np.float32))
    xkT = [b(np.asarray(key, dtype=np.float32)[bb].T) for bb in range(B)]
    xvT = [b(np.asarray(value, dtype=np.float32)[bb].T) for bb in range(B)]
    Wk = np.asarray(Wk, dtype=np.float32)
    Wv = np.asarray(Wv, dtype=np.float32)
    wkT = [b(Wk[HGD * g:HGD * (g + 1), :].T) for g in range(CPB)]
    wvT = [b(Wv[HGD * g:HGD * (g + 1), :].T) for g in range(CPB)]

    in_maps = []
    for c in range(NCORES):
        bb, g = divmod(c, CPB)
        in_maps.append({
            "xqT": b(query[bb, QSL * g:QSL * (g + 1), :].T),
            "xkT": xkT[bb],
            "xvT": xvT[bb],
            "wqT": wqT,
            "wkT": wkT[g],
            "wvT": wvT[g],
            "woT": woT,
            "bo": bo32,
        })
    return in_maps


def assemble(results):
    """Concatenate per-core token slices into [B, NT, D]."""
    out = np.empty((B, NT, D), dtype=np.float32)
    for c in range(NCORES):
        bb, g = divmod(c, CPB)
        out[bb, QSL * g:QSL * (g + 1), :] = results[c]["out"]
    return out


def run(query, key, value, Wq, Wk, Wv, Wo, bo, trace=False):
    from concourse.bass_utils import run_bass_kernel_spmd
    nc = _get_program()
    in_maps = make_in_maps(query, key, value, Wq, Wk, Wv, Wo, bo)
    res = run_bass_kernel_spmd(nc, in_maps, core_ids=list(range(NCORES)),
                               trace=trace)
    return assemble(res.results), res


def kernel(query, key, value, qpos=None, kpos=None, Wq=None, Wk=None,
           Wv=None, Wo=None, bo=None):
    out, _ = run(query, key, value, Wq, Wk, Wv, Wo, bo)
    return out
